# revision 1
# baseline (speedup 1.0000x reference)
"""SKA module Trainium2 kernel: data-parallel over B, head-parallel over H.

8 NeuronCores, 2 heads per core. Two SPMD launches with host linear algebra
in between:

Launch 1 (per core): stream h (fp16, host-pre-transposed to [B, D, T]),
project [z | zq | v] for the core's 2 heads on the PE as a uniform
back-to-back fp16 matmul chain (PSUM fp32 accumulate over the 8 K-chunks),
stage to SBUF, and write z|zq (4MB) + v (4MB) per core.

Host: masked covariances G (gram), M (lag-1 cross), C (value-key), per-token
z-norm max — BLAS sgemm, ~3% of total FLOPs — then the per-(b,h) 32x32
linear algebra (cholesky, 6-step power iteration, triangular solves) in
float64. Everything after the covariances collapses into one matrix per
(b,h):  y[t] = E @ zq[t]  with  E = eta/maxnorm * C_v L^-T A^2 L^-1.

Launch 2 (per core): y^T[hp, t] = Ebd^T @ zq^T on the PE (Ebd stationary,
512-token streams), fp16 out, host un-transposes and upcasts.
"""

import numpy as np
import concourse.bass as bass
import concourse.tile as tile
from concourse import mybir
from concourse.bass_utils import run_bass_kernel_spmd

F32 = mybir.dt.float32
F16 = mybir.dt.float16

B, T, D = 4, 4096, 1024
NCORES = 8
HC = 2          # heads per core
R = 32          # rank
P = 64          # d_head
NCHUNK = T // 128          # 32 t-chunks of 128 per b
TGROUP = 2048              # tokens per h-load group
NGROUP = T // TGROUP
CPG = TGROUP // 128        # chunks per group
RIDGE_EPS = 1e-3


def _legalize_waits(nc, max_waits=1):
    """This walrus build rejects >1 sync wait per instruction; split extras
    into single-wait NoOps spliced before the instruction on its engine."""
    for bb in nc.main_func.blocks:
        new_insts = []
        for inst in bb.instructions:
            si = inst.sync_info
            if si is not None and si.on_wait and len(si.on_wait) > max_waits:
                waits = list(si.on_wait)
                head, tail = waits[:-max_waits], waits[-max_waits:]
                for w in head:
                    nop = mybir.InstNoOp(
                        name=f"I-waitsplit-{nc.next_id()}",
                        engine=inst.engine,
                        ins=[],
                        outs=[],
                        sync_info=mybir.SyncInfo(on_wait=[w], on_update=[]),
                    )
                    nc.register_instruction(nop)
                    new_insts.append(nop)
                inst.sync_info = mybir.SyncInfo(
                    on_wait=tail, on_update=list(si.on_update or [])
                )
            new_insts.append(inst)
        bb.instructions[:] = new_insts


def build_launch1():
    nc = bass.Bass("TRN2")
    h_in = nc.declare_dram_parameter("h16t", [B, D, T], F16, isOutput=False)
    w_in = nc.declare_dram_parameter("wt", [8, 128, 256], F16, isOutput=False)
    zkq_out = nc.declare_dram_parameter("zkq", [B, 128, NCHUNK, 128], F16, isOutput=True)
    v_out = nc.declare_dram_parameter("v", [B, 128, NCHUNK, 128], F16, isOutput=True)

    with tile.TileContext(nc) as tc:
        with (
            tc.tile_pool(name="persist", bufs=1) as persist,
            tc.tile_pool(name="hpool", bufs=2) as hpool,
            tc.tile_pool(name="stage", bufs=2) as stage,
            tc.tile_pool(name="pk_ps", bufs=3, space="PSUM") as pk_ps,
        ):
            wt_sb = persist.tile([128, 8, 256], F16)
            nc.sync.dma_start(out=wt_sb, in_=w_in.rearrange("c p j -> p c j"))

            for b in range(B):
                zkq_stage = stage.tile([128, NCHUNK, 128], F16, tag="zkq")
                v_stage = stage.tile([128, NCHUNK, 128], F16, tag="v")
                for g in range(NGROUP):
                    h_sb = hpool.tile([128, 8, TGROUP], F16, tag="h")
                    for w in range(TGROUP // 512):
                        t0 = g * TGROUP + w * 512
                        nc.sync.dma_start(
                            out=h_sb[:, :, w * 512 : (w + 1) * 512],
                            in_=h_in[b, :, t0 : t0 + 512].rearrange(
                                "(dk p) t -> p dk t", p=128
                            ),
                        )
                    for cc in range(CPG):
                        c = g * CPG + cc
                        pk = pk_ps.tile([128, 256], F32, tag="pk")
                        for dk in range(8):
                            nc.tensor.matmul(
                                pk,
                                h_sb[:, dk, cc * 128 : (cc + 1) * 128],
                                wt_sb[:, dk, :],
                                start=(dk == 0),
                                stop=(dk == 7),
                            )
                        nc.scalar.copy(out=zkq_stage[:, c, :], in_=pk[:, 0:128])
                        nc.vector.tensor_copy(out=v_stage[:, c, :], in_=pk[:, 128:256])
                        if (c + 1) % 16 == 0:
                            qf = c // 16
                            cs = slice(qf * 16, (qf + 1) * 16)
                            nc.sync.dma_start(
                                out=zkq_out[b, :, cs, :], in_=zkq_stage[:, cs, :]
                            )
                            nc.sync.dma_start(
                                out=v_out[b, :, cs, :], in_=v_stage[:, cs, :]
                            )

    _legalize_waits(nc)
    return nc


def build_launch2():
    nc = bass.Bass("TRN2")
    zqt_in = nc.declare_dram_parameter("zqt", [B, 2 * R, T], F16, isOutput=False)
    e_in = nc.declare_dram_parameter("ebd", [B, 2 * R, 2 * P], F16, isOutput=False)
    y_out = nc.declare_dram_parameter("y", [B, 2 * P, T], F16, isOutput=True)

    NW = T // 512

    with tile.TileContext(nc) as tc:
        with (
            tc.tile_pool(name="zqpool", bufs=2) as zqpool,
            tc.tile_pool(name="epool", bufs=2) as epool,
            tc.tile_pool(name="ypool", bufs=2) as ypool,
            tc.tile_pool(name="y_ps", bufs=4, space="PSUM") as y_ps,
        ):
            for b in range(B):
                e_sb = epool.tile([2 * R, 2 * P], F16, tag="e")
                nc.sync.dma_start(out=e_sb, in_=e_in[b])
                zqt_sb = zqpool.tile([2 * R, T], F16, tag="zqt")
                for w in range(2):
                    nc.sync.dma_start(
                        out=zqt_sb[:, w * (T // 2) : (w + 1) * (T // 2)],
                        in_=zqt_in[b, :, w * (T // 2) : (w + 1) * (T // 2)],
                    )
                y_stage = ypool.tile([128, T], F16, tag="ystage")
                for w in range(NW):
                    yp = y_ps.tile([128, 512], F32, tag="y")
                    nc.tensor.matmul(
                        yp,
                        e_sb,
                        zqt_sb[:, w * 512 : (w + 1) * 512],
                        start=True,
                        stop=True,
                    )
                    if w % 2 == 0:
                        nc.scalar.copy(out=y_stage[:, w * 512 : (w + 1) * 512], in_=yp)
                    else:
                        nc.vector.tensor_copy(
                            out=y_stage[:, w * 512 : (w + 1) * 512], in_=yp
                        )
                nc.sync.dma_start(out=y_out[b], in_=y_stage)

    _legalize_waits(nc)
    return nc


def host_prep(h, Wk, Wq, Wv):
    h16 = np.ascontiguousarray(h.transpose(0, 2, 1), dtype=np.float16)
    wts = []
    for core in range(NCORES):
        hs = slice(core * HC * R, (core + 1) * HC * R)
        vs = slice(core * HC * P, (core + 1) * HC * P)
        wcat = np.concatenate([Wk[hs], Wq[hs], Wv[vs]], axis=0)  # [256, 1024]
        wts.append(np.ascontiguousarray(wcat.T.reshape(8, 128, 256), dtype=np.float16))
    return h16, wts


def host_mid(res1, prefix_mask, eta, ssn_gamma):
    """Covariances + per-(b,h) float64 algebra -> launch-2 inputs."""
    eta = float(eta)
    gclip = float(np.clip(ssn_gamma, 1.0, 1.5))
    maskf = prefix_mask.astype(np.float32)
    zqts, ebds = [], []
    for core in range(NCORES):
        zkq = res1[core]["zkq"]
        vv_ = res1[core]["v"]
        zkq_t = zkq.transpose(0, 2, 1, 3).reshape(B, T, 128)
        v_t = vv_.transpose(0, 2, 1, 3).reshape(B, T, 128)[:, :, : 2 * P]
        z = zkq_t[:, :, 0 : 2 * R].astype(np.float32)
        zq = zkq_t[:, :, 2 * R : 4 * R]
        v = v_t.astype(np.float32)

        zqts.append(np.ascontiguousarray(zq.transpose(0, 2, 1)))

        ebd = np.zeros((B, 2 * R, 2 * P), np.float64)
        for b in range(B):
            m = maskf[b][:, None]
            z_m = z[b] * m
            v_m = v[b] * m
            for hh in range(HC):
                rs = slice(hh * R, (hh + 1) * R)
                ps_ = slice(hh * P, (hh + 1) * P)
                zmh = z_m[:, rs]
                G = (zmh.T @ zmh).astype(np.float64)
                M = (zmh[1:].T @ zmh[:-1]).astype(np.float64)
                C = (v_m[:, ps_].T @ zmh).astype(np.float64)
                nrm2 = (z[b][:, rs].astype(np.float64) ** 2).sum(axis=1)
                mn = max(np.sqrt(nrm2.max()), 1e-6)
                imn = 1.0 / mn
                Gs = G * imn * imn + RIDGE_EPS * np.eye(R)
                Gs = 0.5 * (Gs + Gs.T)
                Ms = M * imn * imn
                Cs = C * imn
                L = np.linalg.cholesky(Gs)
                Y = np.linalg.solve(L, Ms)
                A = np.linalg.solve(L, Y.T).T
                pw = np.ones((R, 1)) / np.sqrt(R)
                for _ in range(6):
                    Av = A @ pw
                    u = Av / max(np.linalg.norm(Av), 1e-8)
                    Atu = A.T @ u
                    pw = Atu / max(np.linalg.norm(Atu), 1e-8)
                sig = np.linalg.norm(A @ pw)
                Aw = A / max(sig, 1.0) * gclip
                Af = Aw @ Aw
                S = np.linalg.inv(L)
                E = Cs @ S.T @ Af @ S
                ebd[b, rs, ps_] = (eta * imn) * E.T
        ebds.append(ebd.astype(np.float16))
    return zqts, ebds


def assemble_y(res2):
    y = np.empty((B, T, 16, P), np.float32)
    for core in range(NCORES):
        yt = res2[core]["y"]
        y[:, :, core * HC : (core + 1) * HC, :] = (
            yt.transpose(0, 2, 1).astype(np.float32).reshape(B, T, HC, P)
        )
    return y


_CACHE = {}


def _get_programs():
    if "nc1" not in _CACHE:
        _CACHE["nc1"] = build_launch1()
        _CACHE["nc2"] = build_launch2()
    return _CACHE["nc1"], _CACHE["nc2"]


def kernel(h, prefix_mask, Wk, Wq, Wv, eta, ssn_gamma, _trace=False, _timings=None):
    h = np.asarray(h, dtype=np.float32)
    prefix_mask = np.asarray(prefix_mask)
    Wk = np.asarray(Wk, dtype=np.float32)
    Wq = np.asarray(Wq, dtype=np.float32)
    Wv = np.asarray(Wv, dtype=np.float32)
    eta = float(np.asarray(eta))
    ssn_gamma = float(np.asarray(ssn_gamma))

    nc1, nc2 = _get_programs()
    h16, wts = host_prep(h, Wk, Wq, Wv)
    cores = list(range(NCORES))

    kw1 = dict(_CACHE.get("runkw1", {}))
    r1 = run_bass_kernel_spmd(
        nc1, [{"h16t": h16, "wt": wts[c]} for c in cores], core_ids=cores, **kw1
    )
    zqts, ebds = host_mid(r1.results, prefix_mask, eta, ssn_gamma)
    kw2 = dict(_CACHE.get("runkw2", {}))
    r2 = run_bass_kernel_spmd(
        nc2, [{"zqt": zqts[c], "ebd": ebds[c]} for c in cores], core_ids=cores, **kw2
    )
    if _timings is not None:
        _timings.append((r1.exec_time_ns, r2.exec_time_ns))
    return assemble_y(r2.results)
